# revision 2
# baseline (speedup 1.0000x reference)
"""NodeDropout kernel for 8 trn2 NeuronCores.

out[e] = values[e] * keep[src[e]] * keep[dst[e]],  keep = ~nodes_flag (1M bools).

Per NeuronCore (edges sharded 8 ways data-parallel):
- nodes_flag bit-packed host-side into a 31250-word uint32 table (1M bits),
  replicated into every SBUF partition (~122KB/partition).
- Edge layout: batch of 16384 edges as (q, s) -> partition q in [0,128),
  free s in [0,128). gpsimd.ap_gather consumes group c's (16 partitions)
  index stream position i from idx[16c + i%16, i//16], so a plain [128,128]
  word-index tile gives stream position i = 16s + r the word of edge
  (q=16c+r, s) -- written redundantly to w[16c+p', 16s+r] for all p'.
- Consumption runs on the full redundant tile with free-broadcast operands
  (bp and v broadcast over the r sub-dimension); the result is valid exactly
  on the diagonal r == q%16, which the host selects during unsharding.
  All DMAs are plain <=3-dim APs; all engine ops are full-tile.
"""
import numpy as np
from contextlib import ExitStack

from concourse import bacc, mybir
from concourse import tile
from concourse.bass_utils import run_bass_kernel_spmd

P = 128
N_CORES = 8
E_TOTAL = 20_000_000
E_PER = E_TOTAL // N_CORES          # 2_500_000
NVI = 2048                          # gather indices per 16-partition group
S = NVI // 16                       # 128 edges per partition per batch
BATCH = P * S                       # 16384 edges per batch
NB = -(-E_PER // BATCH)             # 153
E_PAD = NB * BATCH
TWORDS = 31250                      # uint32 words = 1M bits

_NC_CACHE = {}


def _build(nb):
    nc = bacc.Bacc()
    u32 = mybir.dt.uint32
    i16 = mybir.dt.int16
    f32 = mybir.dt.float32

    e_pad = nb * BATCH
    eix = nc.declare_dram_parameter("eix", [2, e_pad, 2], u32, isOutput=False)
    vals = nc.declare_dram_parameter("vals", [e_pad], f32, isOutput=False)
    ktab = nc.declare_dram_parameter("ktab", [P, TWORDS], u32, isOutput=False)
    out = nc.declare_dram_parameter("out", [nb, P, NVI], f32, isOutput=True)

    shr = mybir.AluOpType.logical_shift_right
    band = mybir.AluOpType.bitwise_and
    mult = mybir.AluOpType.mult

    with ExitStack() as ctx:
        tc = ctx.enter_context(tile.TileContext(nc))
        tab_pool = ctx.enter_context(tc.tile_pool(name="tab", bufs=1))
        sm_pool = ctx.enter_context(tc.tile_pool(name="sm", bufs=2))
        w_pool = ctx.enter_context(tc.tile_pool(name="w", bufs=2))

        table_t = tab_pool.tile([P, TWORDS], u32)
        nc.sync.dma_start(table_t[:], ktab[:])

        for b in range(nb):
            lo, hi = b * BATCH, (b + 1) * BATCH

            # low uint32 words of the int64 node ids, edge (q, s) at [q, s]
            ul = sm_pool.tile([P, 2 * S], u32, tag="ul")
            nc.sync.dma_start(ul[:, 0:S], eix[0, lo:hi, 0]
                              .rearrange("(q s) -> q s", s=S))
            nc.sync.dma_start(ul[:, S:2 * S], eix[1, lo:hi, 0]
                              .rearrange("(q s) -> q s", s=S))
            v_t = sm_pool.tile([P, S], f32, tag="v")
            nc.sync.dma_start(v_t[:], vals[lo:hi].rearrange("(q s) -> q s", s=S))

            bp = sm_pool.tile([P, 2 * S], u32, tag="bp")
            nc.vector.tensor_scalar(bp[:], ul[:], 31, None, op0=band)
            wx = sm_pool.tile([P, 2 * S], u32, tag="wx")
            nc.vector.tensor_scalar(wx[:], ul[:], 5, None, op0=shr)
            widx = sm_pool.tile([P, 2 * S], i16, tag="widx")
            nc.vector.tensor_copy(widx[:], wx[:])

            w_s = w_pool.tile([P, NVI], u32, tag="w_s")
            nc.gpsimd.ap_gather(w_s[:], table_t[:], widx[:, 0:S],
                                channels=P, num_elems=TWORDS, d=1, num_idxs=NVI)
            w_d = w_pool.tile([P, NVI], u32, tag="w_d")
            nc.gpsimd.ap_gather(w_d[:], table_t[:], widx[:, S:2 * S],
                                channels=P, num_elems=TWORDS, d=1, num_idxs=NVI)

            # t = w >> bp  (bp broadcast over the r sub-dim; diagonal r==q%16 valid)
            w_s3 = w_s[:].rearrange("q (s r) -> q s r", s=S, r=16)
            w_d3 = w_d[:].rearrange("q (s r) -> q s r", s=S, r=16)
            bp_s3 = bp[:, 0:S].unsqueeze(2).to_broadcast([P, S, 16])
            bp_d3 = bp[:, S:2 * S].unsqueeze(2).to_broadcast([P, S, 16])
            nc.vector.tensor_tensor(w_s3, w_s3, bp_s3, op=shr)
            nc.vector.tensor_tensor(w_d3, w_d3, bp_d3, op=shr)

            # mask = (t_s & 1) & t_d   in {0,1}
            nc.vector.tensor_scalar(w_s[:], w_s[:], 1, None, op0=band)
            nc.vector.tensor_tensor(w_s[:], w_s[:], w_d[:], op=band)

            # mask -> f32 in place (same bytes, converting copy)
            mf = w_s[:].bitcast(f32)
            nc.vector.tensor_copy(mf, w_s[:])
            # out = v * mask (v broadcast over r)
            v3 = v_t[:].unsqueeze(2).to_broadcast([P, S, 16])
            nc.vector.tensor_tensor(mf.rearrange("q (s r) -> q s r", s=S, r=16),
                                    mf.rearrange("q (s r) -> q s r", s=S, r=16),
                                    v3, op=mult)
            nc.sync.dma_start(out[b], mf)
    nc.finalize()
    return nc


def prepare(inputs):
    """Build (nc, in_maps) for the full-problem inputs dict."""
    edge_index = inputs["edge_index"]
    values = inputs["values"]
    nodes_flag = inputs["nodes_flag"]
    e_total = values.shape[0]
    assert e_total % N_CORES == 0
    e_per = e_total // N_CORES
    nb = -(-e_per // BATCH)
    e_pad = nb * BATCH

    if nb not in _NC_CACHE:
        _NC_CACHE[nb] = _build(nb)
    nc = _NC_CACHE[nb]

    keep = ~np.asarray(nodes_flag, dtype=bool)
    keep_pad = np.zeros(TWORDS * 32, dtype=bool)
    keep_pad[:keep.shape[0]] = keep
    ktab_words = np.packbits(keep_pad, bitorder="little").view(np.uint32)
    ktab = np.ascontiguousarray(np.broadcast_to(ktab_words, (P, TWORDS)))

    ei = np.asarray(edge_index)
    vals = np.asarray(values, dtype=np.float32)

    in_maps = []
    for c in range(N_CORES):
        lo, hi = c * e_per, (c + 1) * e_per
        eix_c = np.zeros((2, e_pad), np.int64)
        eix_c[:, :e_per] = ei[:, lo:hi]
        v_c = np.zeros((e_pad,), np.float32)
        v_c[:e_per] = vals[lo:hi]
        in_maps.append({
            "eix": eix_c.view(np.uint32).reshape(2, e_pad, 2),
            "vals": v_c,
            "ktab": ktab,
        })
    return nc, in_maps


def kernel(edge_index: np.ndarray, values: np.ndarray, nodes_flag: np.ndarray) -> np.ndarray:
    e_total = values.shape[0]
    e_per = e_total // N_CORES
    nb = -(-e_per // BATCH)
    e_pad = nb * BATCH
    nc, in_maps = prepare({"edge_index": edge_index, "values": values,
                           "nodes_flag": nodes_flag})

    res = run_bass_kernel_spmd(nc, in_maps, list(range(N_CORES)))

    # diagonal select r == q%16, then (q, s) -> flat edge order
    rsel = (np.arange(P) % 16)[None, :, None, None]
    outs = []
    for c in range(N_CORES):
        o = res.results[c]["out"].reshape(nb, P, S, 16)
        o = np.take_along_axis(o, rsel, axis=3)[..., 0]    # [nb, P, S]
        outs.append(o.reshape(e_pad)[:e_per])
    return np.concatenate(outs).astype(np.float32)


if __name__ == "__main__":
    import sys
    rng = np.random.default_rng(0)
    nbatches = int(sys.argv[1]) if len(sys.argv) > 1 else 8
    E = BATCH * nbatches * N_CORES
    N = 1_000_000
    ei = rng.integers(0, N, size=(2, E), dtype=np.int64)
    v = rng.random(E, dtype=np.float32)
    flag = rng.random(N) < 0.1
    got = kernel(ei, v, flag)
    keep = (~flag).astype(np.float32)
    exp = v * keep[ei[0]] * keep[ei[1]]
    err = np.max(np.abs(got - exp))
    print("max abs err:", err, "CORRECT:", np.allclose(got, exp))

